# revision 1
# baseline (speedup 1.0000x reference)
"""Causal self-attention (B=2, T=2048, E=1024, H=16, D=64) on 8 NeuronCores.

Sharding: batch (2) x head-groups (4 groups of 4 heads) -> 8 cores.
Each core computes, for its (batch b, head group g):
  Q^T/K^T = (Wq/Wk col-shard)^T @ x_b^T        (heads on partitions, t free)
  V       = x_b @ Wv col-shard                 (tokens on partitions)
  S^T     = K^T-slices^T-matmuls (s on partitions, t free), causal
  P^T     = exp(S^T/8) (no max subtraction: scores ~ N(0,1), exp is safe)
  U^T,r   = [V | ones]^T @ P^T  (PV product + row-sums in one matmul)
  A^T     = U^T * (1/r)                        (softmax normalization)
  Ypart   = A^T-slices^T @ Wo row-shard        (partial out-proj)
Host sums the 4 partials per batch and adds the constant row
bo + bv @ Wo (exact: v-bias passes through attention unchanged; the k-bias
is softmax-invariant and dropped; q-bias is applied to Q on device).

Matmuls run in float32r (single-pass reduced-precision fp32, ~2e-4 rel err).
"""
import os
import sys

if "/opt/trn_rl_repo" not in sys.path:
    sys.path.insert(0, "/opt/trn_rl_repo")

import numpy as np

import concourse.bass as bass
import concourse.mybir as mybir
import concourse.tile as tile
from concourse import bacc
from concourse.bass_utils import run_bass_kernel_spmd

F32 = mybir.dt.float32
F32R = mybir.dt.float32r

B, T, E = 2, 2048, 1024
H, D = 16, 64
N_CORES = 8
HEADS_PER_CORE = 4            # 16 heads / 4 groups
EPC = HEADS_PER_CORE * D      # 256: e' columns per core
TC = 512                      # t-chunk (psum free width)
NTC = T // TC                 # 4 t-chunks
SB = 128                      # s-block (psum partitions)
NSB = T // SB                 # 16 s-blocks
KCH = E // 128                # 8 contraction chunks


def build_kernel(repeat: int = 1) -> bass.Bass:
    """repeat>1 wraps the whole compute in a hardware loop — used only for
    wall-clock timing (the axon round-trip is ~91ms, so per-iteration time
    is measured as (wall(R) - wall(1)) / (R - 1))."""
    nc = bacc.Bacc(None, target_bir_lowering=False, debug=False)

    xT = nc.dram_tensor("xT", [E, T], F32, kind="ExternalInput")
    wq = nc.dram_tensor("wq", [E, EPC], F32, kind="ExternalInput")
    wk = nc.dram_tensor("wk", [E, EPC], F32, kind="ExternalInput")
    wv = nc.dram_tensor("wv", [E, EPC], F32, kind="ExternalInput")
    wo = nc.dram_tensor("wo", [EPC, E], F32, kind="ExternalInput")
    bq = nc.dram_tensor("bq", [EPC], F32, kind="ExternalInput")
    onesc = nc.dram_tensor("onesc", [128, NSB * HEADS_PER_CORE], F32,
                           kind="ExternalInput")
    y = nc.dram_tensor("y", [T, E], F32, kind="ExternalOutput")

    with tile.TileContext(nc) as tc:
        with tc.tile_pool(name="singles", bufs=1) as singles, \
             tc.tile_pool(name="xtp", bufs=3) as xtp, \
             tc.tile_pool(name="pp", bufs=6) as pp, \
             tc.tile_pool(name="rp", bufs=4) as rp, \
             tc.tile_pool(name="rbp", bufs=4) as rbp, \
             tc.tile_pool(name="ysb", bufs=4) as ysbp, \
             tc.tile_pool(name="ps_a", bufs=2, space="PSUM") as ps_a, \
             tc.tile_pool(name="ps_s", bufs=2, space="PSUM") as ps_s, \
             tc.tile_pool(name="ps_u", bufs=2, space="PSUM") as ps_u:

            # ---- weight / bias loads (once) ----
            wq_sb = singles.tile([128, KCH, EPC], F32R, tag="wq")
            wk_sb = singles.tile([128, KCH, EPC], F32R, tag="wk")
            wv_sb = singles.tile([128, KCH, EPC], F32R, tag="wv")
            for k in range(KCH):
                nc.sync.dma_start(out=wq_sb[:, k, :],
                                  in_=wq[k * 128:(k + 1) * 128, :].bitcast(F32R))
                nc.sync.dma_start(out=wk_sb[:, k, :],
                                  in_=wk[k * 128:(k + 1) * 128, :].bitcast(F32R))
                nc.sync.dma_start(out=wv_sb[:, k, :],
                                  in_=wv[k * 128:(k + 1) * 128, :].bitcast(F32R))
            # wo: head h lives at partitions 64*(h%2).. of slab h//2
            wo_sb = singles.tile([128, 2, E], F32R, tag="wo")
            for j in range(2):
                nc.sync.dma_start(out=wo_sb[:, j, :],
                                  in_=wo[j * 128:(j + 1) * 128, :].bitcast(F32R))
            bq_sb = singles.tile([128, 2], F32, tag="bq")
            for eh in range(2):
                nc.sync.dma_start(out=bq_sb[:, eh],
                                  in_=bq[eh * 128:(eh + 1) * 128])

            # ---- persistent activations ----
            # QT/KT/AT: pair slab eh holds heads (2eh, 2eh+1) on partition
            # halves; V_all[s_part, s_block, head, 0:64]=V, [.,.,.,64]=1.0
            QT = [singles.tile([128, T], F32R, tag=f"QT{eh}", name=f"QT{eh}") for eh in range(2)]
            KT = [singles.tile([128, T], F32R, tag=f"KT{eh}", name=f"KT{eh}") for eh in range(2)]
            AT = [singles.tile([128, T], F32R, tag=f"AT{eh}", name=f"AT{eh}") for eh in range(2)]
            V_all = singles.tile([128, NSB, HEADS_PER_CORE, D + 1], F32R, tag="V")
            # ones column (memset doesn't support f32r: DMA a constant in)
            nc.sync.dma_start(out=V_all[:, :, :, D:D + 1],
                              in_=onesc[:].bitcast(F32R))

            def emit_body():
                for c in range(NTC):
                    t0 = c * TC
                    # ======== phase A: projections for t-chunk c ========
                    xt = xtp.tile([128, KCH, TC], F32R, tag="xt")
                    for k in range(KCH):
                        nc.sync.dma_start(
                            out=xt[:, k, :],
                            in_=xT[k * 128:(k + 1) * 128, t0:t0 + TC].bitcast(F32R))

                    for eh in range(2):
                        q_ps = ps_a.tile([128, TC], F32, tag="a")
                        for k in range(KCH):
                            nc.tensor.matmul(
                                q_ps[:], wq_sb[:, k, eh * 128:(eh + 1) * 128],
                                xt[:, k, :], start=(k == 0), stop=(k == KCH - 1))
                        nc.vector.tensor_scalar_add(
                            out=QT[eh][:, t0:t0 + TC], in0=q_ps[:],
                            scalar1=bq_sb[:, eh:eh + 1])

                        k_ps = ps_a.tile([128, TC], F32, tag="a")
                        for k in range(KCH):
                            nc.tensor.matmul(
                                k_ps[:], wk_sb[:, k, eh * 128:(eh + 1) * 128],
                                xt[:, k, :], start=(k == 0), stop=(k == KCH - 1))
                        nc.vector.tensor_copy(KT[eh][:, t0:t0 + TC], k_ps[:])

                    for j4 in range(TC // SB):
                        v_ps = ps_a.tile([128, EPC], F32, tag="a")
                        for k in range(KCH):
                            nc.tensor.matmul(
                                v_ps[:], xt[:, k, j4 * SB:(j4 + 1) * SB],
                                wv_sb[:, k, :], start=(k == 0), stop=(k == KCH - 1))
                        nc.vector.tensor_copy(
                            V_all[:, c * (TC // SB) + j4, :, 0:D],
                            v_ps[:].rearrange("p (h d) -> p h d", h=HEADS_PER_CORE))

                    if "B" not in os.environ.get("K_PHASES", "ABC"):
                        continue
                    # ======== phase B: attention for t-chunk c ========
                    # waves of head-pairs; per (j, wave) one grouped exp over
                    # both heads; diagonal blocks narrowed to valid columns
                    nblk = (c + 1) * (TC // SB)
                    for eh in range(2):
                        u_pair = [ps_u.tile([D + 1, TC], F32, tag="u",
                                            name=f"u{c}_{eh}_{h2}")
                                  for h2 in range(2)]
                        for j in range(nblk):
                            # valid columns of this t-chunk: t >= j*SB
                            off = max(0, j * SB - t0)
                            w = TC - off
                            s2 = ps_s.tile([128, 2, TC], F32, tag="s2")
                            for h2 in range(2):
                                r0 = 64 * h2
                                nc.tensor.matmul(
                                    s2[:, h2, off:],
                                    KT[eh][r0:r0 + 64, j * SB:(j + 1) * SB],
                                    QT[eh][r0:r0 + 64, t0 + off:t0 + TC],
                                    start=True, stop=True)
                            p2 = pp.tile([128, 2, TC], F32R, tag="pj")
                            nc.scalar.activation(
                                p2[:, :, off:], s2[:, :, off:],
                                mybir.ActivationFunctionType.Exp, scale=0.125)
                            if j >= c * (TC // SB):
                                # triangle: keep where (t0+off+y) >= (j*SB+x),
                                # same pattern for both heads (middle dim 0-step)
                                nc.gpsimd.affine_select(
                                    out=p2[:, :, off:], in_=p2[:, :, off:],
                                    compare_op=mybir.AluOpType.is_ge, fill=0.0,
                                    base=t0 + off - j * SB, pattern=[[0, 2], [1, w]],
                                    channel_multiplier=-1)
                            for h2 in range(2):
                                nc.tensor.matmul(
                                    u_pair[h2][:, off:],
                                    V_all[:, j, 2 * eh + h2, :], p2[:, h2, off:],
                                    start=(j == 0), stop=(j == nblk - 1))

                        for h2 in range(2):
                            r0 = 64 * h2
                            r_row = rp.tile([1, TC], F32, tag="rr")
                            nc.vector.tensor_copy(r_row[:], u_pair[h2][D:D + 1, :])
                            rinv = rp.tile([1, TC], F32, tag="ri")
                            nc.vector.reciprocal_approx_fast(out=rinv[:], in_=r_row[:])
                            rb = rbp.tile([64, TC], F32, tag="rb")
                            nc.gpsimd.partition_broadcast(rb[:], rinv[:])
                            nc.vector.tensor_mul(
                                AT[eh][r0:r0 + 64, t0:t0 + TC],
                                u_pair[h2][0:D, :], rb[:])

                    if "C" not in os.environ.get("K_PHASES", "ABC"):
                        continue
                    # ======== phase C: out-proj for this chunk's t-blocks ========
                    for tb4 in range(TC // SB):
                        tb0 = t0 + tb4 * SB
                        for e in range(2):
                            y_ps = ps_a.tile([128, 512], F32, tag="a")
                            # contraction over e' = pair-slab partitions:
                            # one K=128 matmul per slab (2 heads at once)
                            for eh in range(2):
                                nc.tensor.matmul(
                                    y_ps[:],
                                    AT[eh][:, tb0:tb0 + SB],
                                    wo_sb[:, eh, e * 512:(e + 1) * 512],
                                    start=(eh == 0), stop=(eh == 1))
                            y_sb = ysbp.tile([128, 512], F32, tag="ysb")
                            nc.vector.tensor_copy(y_sb[:], y_ps[:])
                            nc.sync.dma_start(
                                out=y[tb0:tb0 + SB, e * 512:(e + 1) * 512],
                                in_=y_sb[:])

            if repeat == 1:
                emit_body()
            else:
                with tc.For_i(0, repeat, 1):
                    emit_body()

    nc.compile()
    return nc


_NC_CACHE = {}


def _get_nc(repeat: int = 1):
    if repeat not in _NC_CACHE:
        _NC_CACHE[repeat] = build_kernel(repeat)
    return _NC_CACHE[repeat]


def make_in_maps(inputs: dict) -> list:
    x = np.asarray(inputs["x"], dtype=np.float32)
    Wq = np.asarray(inputs["Wq"], dtype=np.float32)
    Wk = np.asarray(inputs["Wk"], dtype=np.float32)
    Wv = np.asarray(inputs["Wv"], dtype=np.float32)
    Wo = np.asarray(inputs["Wo"], dtype=np.float32)
    bq = np.asarray(inputs["bq"], dtype=np.float32)

    in_maps = []
    for core in range(N_CORES):
        b, g = divmod(core, N_CORES // B)
        cs = slice(g * EPC, (g + 1) * EPC)
        in_maps.append({
            "xT": np.ascontiguousarray(x[b].T),
            "wq": np.ascontiguousarray(Wq[:, cs]),
            "wk": np.ascontiguousarray(Wk[:, cs]),
            "wv": np.ascontiguousarray(Wv[:, cs]),
            "wo": np.ascontiguousarray(Wo[cs, :]),
            "bq": np.ascontiguousarray(bq[cs]),
            "onesc": np.ones((128, NSB * HEADS_PER_CORE), dtype=np.float32),
        })
    return in_maps


def run_sharded(inputs: dict, trace: bool = False):
    """Shard inputs, run the SPMD kernel on 8 cores, unshard. Returns
    (output (B,T,E) float32, BassKernelResults)."""
    Wo = np.asarray(inputs["Wo"], dtype=np.float32)
    bv = np.asarray(inputs["bv"], dtype=np.float32)
    bo = np.asarray(inputs["bo"], dtype=np.float32)

    in_maps = make_in_maps(inputs)
    res = run_bass_kernel_spmd(_get_nc(), in_maps, core_ids=list(range(N_CORES)),
                               trace=trace)

    # unshard: sum the 4 head-group partials per batch; add the constant row
    # bo + bv @ Wo (v-bias commutes through the attention average exactly).
    const_row = (bo.astype(np.float64)
                 + bv.astype(np.float64) @ Wo.astype(np.float64))
    out = np.empty((B, T, E), dtype=np.float32)
    for b in range(B):
        acc = np.zeros((T, E), dtype=np.float64)
        for g in range(N_CORES // B):
            acc += res.results[b * (N_CORES // B) + g]["y"].astype(np.float64)
        out[b] = (acc + const_row).astype(np.float32)
    return out, res


def kernel(**inputs) -> np.ndarray:
    out, _ = run_sharded(inputs, trace=False)
    return out



# revision 2
# speedup vs baseline: 2.0498x; 2.0498x over previous
"""Causal self-attention (B=2, T=2048, E=1024, H=16, D=64) on 8 NeuronCores.

Sharding: batch (2) x head-groups (4 groups of 4 heads) -> 8 cores.
Each core computes, for its (batch b, head group g):
  Q^T/K^T = (Wq/Wk col-shard)^T @ x_b^T        (heads on partitions, t free)
  V       = x_b @ Wv col-shard                 (tokens on partitions)
  S^T     = K^T-slices^T-matmuls (s on partitions, t free), causal
  P^T     = exp(S^T/8) (no max subtraction: scores ~ N(0,1), exp is safe)
  U^T,r   = [V | ones]^T @ P^T  (PV product + row-sums in one matmul)
  A^T     = U^T * (1/r)                        (softmax normalization)
  Ypart   = A^T-slices^T @ Wo row-shard        (partial out-proj)
Host sums the 4 partials per batch and adds the constant row
bo + bv @ Wo (exact: v-bias passes through attention unchanged; the k-bias
is softmax-invariant and dropped; q-bias is applied to Q on device).

v2: all matmul operands in bf16 (PSUM accumulation stays f32; total rel
err ~1e-3 vs the 2e-2 gate). Phase B is software-pipelined per (chunk,
head-pair slab): S matmuls run LAG steps ahead of the U matmuls that
consume exp(S), and projection/out-proj chains from neighboring chunks
are interleaved as PE filler, paced by an emission-time cost estimator,
so the tensor engine never stalls on the scalar-engine exp chain.
"""
import sys

if "/opt/trn_rl_repo" not in sys.path:
    sys.path.insert(0, "/opt/trn_rl_repo")

import ml_dtypes
import numpy as np

import concourse.bass as bass
import concourse.mybir as mybir
import concourse.tile as tile
from concourse import bacc
from concourse.bass_utils import run_bass_kernel_spmd

F32 = mybir.dt.float32
BF16 = mybir.dt.bfloat16
NP_BF16 = ml_dtypes.bfloat16

B, T, E = 2, 2048, 1024
H, D = 16, 64
N_CORES = 8
HEADS_PER_CORE = 4            # 16 heads / 4 groups
EPC = HEADS_PER_CORE * D      # 256: e' columns per core
TC = 512                      # t-chunk (psum free width)
NTC = T // TC                 # 4 t-chunks
SB = 128                      # s-block (psum partitions)
NSB = T // SB                 # 16 s-blocks
KCH = E // 128                # 8 contraction chunks
LAG = 3                       # U lags S by LAG j-steps in phase B

# emission-time cost estimates (ns) for the pacing scheduler
PE_NS_PER_COL = 1.0 / 2.4
ACT_NS_PER_EL = 1.0 / 1.2
ACT_OVH = 175.0
EXP_LAT = 600.0               # sem + access latency margin S->exp->U


def build_kernel(repeat: int = 1) -> bass.Bass:
    """repeat>1 wraps the whole compute in a hardware loop — used only for
    wall-clock timing (the axon round-trip is ~91ms, so per-iteration time
    is measured as (wall(R) - wall(1)) / (R - 1))."""
    nc = bacc.Bacc(None, target_bir_lowering=False, debug=False)

    xT = nc.dram_tensor("xT", [E, T], BF16, kind="ExternalInput")
    wq = nc.dram_tensor("wq", [E, EPC], BF16, kind="ExternalInput")
    wk = nc.dram_tensor("wk", [E, EPC], BF16, kind="ExternalInput")
    wv = nc.dram_tensor("wv", [E, EPC], BF16, kind="ExternalInput")
    wo = nc.dram_tensor("wo", [EPC, E], BF16, kind="ExternalInput")
    bq = nc.dram_tensor("bq", [EPC], F32, kind="ExternalInput")
    onesc = nc.dram_tensor("onesc", [128, NSB * HEADS_PER_CORE], BF16,
                           kind="ExternalInput")
    y = nc.dram_tensor("y", [T, E], F32, kind="ExternalOutput")

    with tile.TileContext(nc) as tc:
        with tc.tile_pool(name="singles", bufs=1) as singles, \
             tc.tile_pool(name="xtp", bufs=3) as xtp, \
             tc.tile_pool(name="rp", bufs=2) as rp, \
             tc.tile_pool(name="rbp", bufs=2) as rbp, \
             tc.tile_pool(name="ysb", bufs=3) as ysbp, \
             tc.tile_pool(name="ps_s", bufs=2, space="PSUM") as ps_s, \
             tc.tile_pool(name="ps_u", bufs=1, space="PSUM") as ps_u, \
             tc.tile_pool(name="ps_a", bufs=2, space="PSUM") as ps_a:

            # ---- weight / bias loads (once) ----
            wq_sb = singles.tile([128, KCH, EPC], BF16, tag="wq")
            wk_sb = singles.tile([128, KCH, EPC], BF16, tag="wk")
            wv_sb = singles.tile([128, KCH, EPC], BF16, tag="wv")
            for k in range(KCH):
                nc.sync.dma_start(out=wq_sb[:, k, :],
                                  in_=wq[k * 128:(k + 1) * 128, :])
                nc.sync.dma_start(out=wk_sb[:, k, :],
                                  in_=wk[k * 128:(k + 1) * 128, :])
                nc.sync.dma_start(out=wv_sb[:, k, :],
                                  in_=wv[k * 128:(k + 1) * 128, :])
            # wo: head h lives at partitions 64*(h%2).. of slab h//2
            wo_sb = singles.tile([128, 2, E], BF16, tag="wo")
            for j in range(2):
                nc.sync.dma_start(out=wo_sb[:, j, :],
                                  in_=wo[j * 128:(j + 1) * 128, :])
            bq_sb = singles.tile([128, 2], F32, tag="bq")
            for eh in range(2):
                nc.sync.dma_start(out=bq_sb[:, eh],
                                  in_=bq[eh * 128:(eh + 1) * 128])

            # ---- persistent activations ----
            # QT/KT/AT: pair slab eh holds heads (2eh, 2eh+1) on partition
            # halves; V_all[s_part, s_block, head, 0:64]=V, [.,.,.,64]=1.0
            QT = [singles.tile([128, T], BF16, tag=f"QT{eh}", name=f"QT{eh}")
                  for eh in range(2)]
            KT = [singles.tile([128, T], BF16, tag=f"KT{eh}", name=f"KT{eh}")
                  for eh in range(2)]
            AT = [singles.tile([128, T], BF16, tag=f"AT{eh}", name=f"AT{eh}")
                  for eh in range(2)]
            V_all = singles.tile([128, NSB, HEADS_PER_CORE, D + 1], BF16,
                                 tag="V")
            nc.sync.dma_start(out=V_all[:, :, :, D:D + 1], in_=onesc[:])
            # P^T staging per slab: [s_part, s_block j, h2, t]
            PF = [singles.tile([128, NSB, 2, TC], BF16, tag=f"PF{eh}",
                               name=f"PF{eh}") for eh in range(2)]

            def emit_body():
                xt_tiles = {}

                def emit_xt_dma(c):
                    xt = xtp.tile([128, KCH, TC], BF16, tag="xt",
                                  name=f"xt{c}")
                    t0 = c * TC
                    for k in range(KCH):
                        nc.sync.dma_start(
                            out=xt[:, k, :],
                            in_=xT[k * 128:(k + 1) * 128, t0:t0 + TC])
                    xt_tiles[c] = xt

                def a_chains(c):
                    """Projection chains for chunk c (each: 8 matmuls + one
                    DVE copy). Returns list of (thunk, pe_cost_ns)."""
                    t0 = c * TC
                    chains = []

                    def qk_chain(eh, which):
                        def go():
                            xt = xt_tiles[c]
                            w_sb = wq_sb if which == "q" else wk_sb
                            ps = ps_a.tile([128, TC], F32, tag="a",
                                           name=f"{which}ps{c}_{eh}")
                            for k in range(KCH):
                                nc.tensor.matmul(
                                    ps[:],
                                    w_sb[:, k, eh * 128:(eh + 1) * 128],
                                    xt[:, k, :],
                                    start=(k == 0), stop=(k == KCH - 1))
                            if which == "q":
                                nc.vector.tensor_scalar_add(
                                    out=QT[eh][:, t0:t0 + TC], in0=ps[:],
                                    scalar1=bq_sb[:, eh:eh + 1])
                            else:
                                nc.vector.tensor_copy(
                                    KT[eh][:, t0:t0 + TC], ps[:])
                        return go

                    def v_chain(j4):
                        def go():
                            xt = xt_tiles[c]
                            ps = ps_a.tile([128, EPC], F32, tag="a",
                                           name=f"vps{c}_{j4}")
                            for k in range(KCH):
                                nc.tensor.matmul(
                                    ps[:], xt[:, k, j4 * SB:(j4 + 1) * SB],
                                    wv_sb[:, k, :],
                                    start=(k == 0), stop=(k == KCH - 1))
                            nc.vector.tensor_copy(
                                V_all[:, c * (TC // SB) + j4, :, 0:D],
                                ps[:].rearrange("p (h d) -> p h d",
                                                h=HEADS_PER_CORE))
                        return go

                    for eh in range(2):
                        chains.append((qk_chain(eh, "q"),
                                       KCH * TC * PE_NS_PER_COL))
                        chains.append((qk_chain(eh, "k"),
                                       KCH * TC * PE_NS_PER_COL))
                    for j4 in range(TC // SB):
                        chains.append((v_chain(j4),
                                       KCH * EPC * PE_NS_PER_COL))
                    return chains

                def c_chains(c):
                    """Out-proj chains for chunk c (each: 2 matmuls +
                    copy + DMA)."""
                    t0 = c * TC
                    chains = []

                    def yc(tb4, e):
                        def go():
                            tb0 = t0 + tb4 * SB
                            y_ps = ps_a.tile([128, TC], F32, tag="a",
                                             name=f"yps{c}_{tb4}_{e}")
                            for eh in range(2):
                                nc.tensor.matmul(
                                    y_ps[:],
                                    AT[eh][:, tb0:tb0 + SB],
                                    wo_sb[:, eh, e * 512:(e + 1) * 512],
                                    start=(eh == 0), stop=(eh == 1))
                            y_sb = ysbp.tile([128, TC], F32, tag="ysb",
                                             name=f"ysb{c}_{tb4}_{e}")
                            nc.vector.tensor_copy(y_sb[:], y_ps[:])
                            nc.sync.dma_start(
                                out=y[tb0:tb0 + SB, e * 512:(e + 1) * 512],
                                in_=y_sb[:])
                        return go

                    for tb4 in range(TC // SB):
                        for e in range(2):
                            chains.append((yc(tb4, e),
                                           2 * 512 * PE_NS_PER_COL))
                    return chains

                def emit_B(c, fillers):
                    """Phase B for chunk c: per eh slab, a j-loop with S
                    matmuls LAG steps ahead of U matmuls; fillers (chains
                    from phases A/C of neighbor chunks) are interleaved to
                    keep PE busy while the scalar engine runs exp."""
                    t0 = c * TC
                    nblk = (c + 1) * (TC // SB)
                    est_pe = 0.0
                    est_act = 0.0
                    exp_done = {}
                    todo = list(fillers)

                    def fill_one():
                        nonlocal est_pe
                        if not todo:
                            return False
                        thunk, cost = todo.pop(0)
                        thunk()
                        est_pe += cost
                        return True

                    def offw(j):
                        off = max(0, j * SB - t0)
                        return off, TC - off

                    for eh in range(2):
                        u_pair = [ps_u.tile([D + 1, TC], F32, tag=f"u{h2}",
                                            name=f"u{c}_{eh}_{h2}")
                                  for h2 in range(2)]
                        for js in range(nblk + LAG):
                            j = js - LAG
                            if j >= 0:
                                # U for block j: pace fillers so exp(j) is
                                # done before PE reaches these matmuls
                                while (est_pe < exp_done[(eh, j)] + EXP_LAT
                                       and fill_one()):
                                    pass
                                off, w = offw(j)
                                for h2 in range(2):
                                    nc.tensor.matmul(
                                        u_pair[h2][:, off:],
                                        V_all[:, j, 2 * eh + h2, :],
                                        PF[eh][:, j, h2, off:],
                                        start=(j == 0), stop=(j == nblk - 1))
                                est_pe += 2 * w * PE_NS_PER_COL
                            if js < nblk:
                                off, w = offw(js)
                                s2 = ps_s.tile([128, 2, TC], F32, tag="s2",
                                               name=f"s2_{c}_{eh}_{js}")
                                for h2 in range(2):
                                    r0 = 64 * h2
                                    nc.tensor.matmul(
                                        s2[:, h2, off:],
                                        KT[eh][r0:r0 + 64,
                                               js * SB:(js + 1) * SB],
                                        QT[eh][r0:r0 + 64,
                                               t0 + off:t0 + TC],
                                        start=True, stop=True)
                                est_pe += 2 * w * PE_NS_PER_COL
                                nc.scalar.activation(
                                    PF[eh][:, js, :, off:], s2[:, :, off:],
                                    mybir.ActivationFunctionType.Exp,
                                    scale=0.125)
                                est_act = (max(est_act, est_pe + 250.0)
                                           + 2 * w * ACT_NS_PER_EL + ACT_OVH)
                                exp_done[(eh, js)] = est_act
                                if js >= c * (TC // SB):
                                    # causal mask on the 128-wide diagonal
                                    # strip (cols beyond it are all-valid)
                                    nc.gpsimd.affine_select(
                                        out=PF[eh][:, js, :, off:off + SB],
                                        in_=PF[eh][:, js, :, off:off + SB],
                                        compare_op=mybir.AluOpType.is_ge,
                                        fill=0.0, base=0,
                                        pattern=[[0, 2], [1, SB]],
                                        channel_multiplier=-1)
                        # softmax normalization for this slab
                        for h2 in range(2):
                            r0 = 64 * h2
                            r_row = rp.tile([1, TC], F32, tag="rr")
                            nc.vector.tensor_copy(r_row[:],
                                                  u_pair[h2][D:D + 1, :])
                            rinv = rp.tile([1, TC], F32, tag="ri")
                            nc.vector.reciprocal_approx_fast(out=rinv[:],
                                                             in_=r_row[:])
                            rb = rbp.tile([64, TC], F32, tag="rb")
                            nc.gpsimd.partition_broadcast(rb[:], rinv[:])
                            nc.vector.tensor_mul(
                                AT[eh][r0:r0 + 64, t0:t0 + TC],
                                u_pair[h2][0:D, :], rb[:])
                    # leftover fillers
                    while fill_one():
                        pass

                # ---- top-level schedule ----
                emit_xt_dma(0)
                emit_xt_dma(1)
                for thunk, _ in a_chains(0):
                    thunk()
                for c in range(NTC):
                    if c + 2 < NTC:
                        emit_xt_dma(c + 2)
                    fillers = []
                    if c + 1 < NTC:
                        fillers += a_chains(c + 1)
                    if c - 1 >= 0:
                        fillers += c_chains(c - 1)
                    emit_B(c, fillers)
                for thunk, _ in c_chains(NTC - 1):
                    thunk()

            if repeat == 1:
                emit_body()
            else:
                with tc.For_i(0, repeat, 1):
                    emit_body()

    nc.compile()
    return nc


_NC_CACHE = {}


def _get_nc(repeat: int = 1):
    if repeat not in _NC_CACHE:
        _NC_CACHE[repeat] = build_kernel(repeat)
    return _NC_CACHE[repeat]


def make_in_maps(inputs: dict) -> list:
    x = np.asarray(inputs["x"], dtype=np.float32)
    Wq = np.asarray(inputs["Wq"], dtype=np.float32)
    Wk = np.asarray(inputs["Wk"], dtype=np.float32)
    Wv = np.asarray(inputs["Wv"], dtype=np.float32)
    Wo = np.asarray(inputs["Wo"], dtype=np.float32)
    bq = np.asarray(inputs["bq"], dtype=np.float32)

    in_maps = []
    for core in range(N_CORES):
        b, g = divmod(core, N_CORES // B)
        cs = slice(g * EPC, (g + 1) * EPC)
        in_maps.append({
            "xT": np.ascontiguousarray(x[b].T).astype(NP_BF16),
            "wq": np.ascontiguousarray(Wq[:, cs]).astype(NP_BF16),
            "wk": np.ascontiguousarray(Wk[:, cs]).astype(NP_BF16),
            "wv": np.ascontiguousarray(Wv[:, cs]).astype(NP_BF16),
            "wo": np.ascontiguousarray(Wo[cs, :]).astype(NP_BF16),
            "bq": np.ascontiguousarray(bq[cs]),
            "onesc": np.ones((128, NSB * HEADS_PER_CORE), dtype=NP_BF16),
        })
    return in_maps


def run_sharded(inputs: dict, trace: bool = False):
    """Shard inputs, run the SPMD kernel on 8 cores, unshard. Returns
    (output (B,T,E) float32, BassKernelResults)."""
    Wo = np.asarray(inputs["Wo"], dtype=np.float32)
    bv = np.asarray(inputs["bv"], dtype=np.float32)
    bo = np.asarray(inputs["bo"], dtype=np.float32)

    in_maps = make_in_maps(inputs)
    res = run_bass_kernel_spmd(_get_nc(), in_maps, core_ids=list(range(N_CORES)),
                               trace=trace)

    # unshard: sum the 4 head-group partials per batch; add the constant row
    # bo + bv @ Wo (v-bias commutes through the attention average exactly).
    const_row = (bo.astype(np.float64)
                 + bv.astype(np.float64) @ Wo.astype(np.float64))
    out = np.empty((B, T, E), dtype=np.float32)
    for b in range(B):
        acc = np.zeros((T, E), dtype=np.float64)
        for g in range(N_CORES // B):
            acc += res.results[b * (N_CORES // B) + g]["y"].astype(np.float64)
        out[b] = (acc + const_row).astype(np.float32)
    return out, res


def kernel(**inputs) -> np.ndarray:
    out, _ = run_sharded(inputs, trace=False)
    return out
